# revision 1
# baseline (speedup 1.0000x reference)
"""BatchedKiloNeRF Trainium2 kernel.

Strategy (expert-parallel, host routing):
  - 4096 tiny MLPs ("experts"), 131072 points routed by model_indices.
  - Host sorts experts by point count, packs them into groups of 16
    (8 cores x 32 groups x 16 experts). Each group of 16 experts maps onto
    the 16 independent 32x32 sub-arrays of the PE (tile_position packing):
    expert (row r, col c) reads rhs from partition band 32r and writes
    PSUM band 32c, so all 16 per-expert matmuls run concurrently.
  - HW constraint (found empirically): concurrent matmuls from different
    row groups must not write the same PSUM bank. Each step therefore uses
    a 4-bank PSUM tile [128, 2048]; row group r writes its outputs into
    bank r (column window 512*r..512*r+C). Copies back to SBUF use strided
    APs so each step needs only one ACT/DVE op.
  - Points of each expert are padded to the group capacity C (max count in
    the 128-expert window); hidden states live as [128, 4C] SBUF tiles:
    partition band = expert band, C-column segment = expert segment.
  - Layer chain alternates layouts: A (band=e%4, seg=e//4) <-> B
    (band=e//4, seg=e%4); every step uses all 16 (row, col) positions.
  - Biases: L0 and view-layer biases ride in the matmul via a constant-1
    input row (K=3 -> K=4). feat bias is folded into the view bias on the
    host. L1 bias+relu is a fused DVE tensor_scalar (add, max) per bank
    slot. sigma/rgb biases are added on host.
  - Weights are pre-permuted/transposed on host into per-group SBUF-layout
    blobs so each group needs only a handful of large DMAs.
"""

import sys

import numpy as np

for _p in ("/opt/trn_rl_repo",):
    if _p not in sys.path:
        sys.path.append(_p)

NUM_MODELS = 4096
W = 32
N = 131072
NCORES = 8
NGROUPS = 32          # expert groups per core
EPG = 16              # experts per group
WIN = NCORES * EPG    # experts per capacity window (128)

# wblob column layout (per group, [128, WBLOB_F]):
#   w1 lhsT    [  0:128)
#   viewA lhsT [128:256)   (view_w[:, :32] @ feat_w folded on host)
#   sigma lhsT [256:260)
#   rgb lhsT   [260:272)
#   b1 bias    [272:276)   (per-slot per-partition bias columns)
#   zero pad   [276:304)   (SIM_SAFE mode widens sigma/rgb lhsT windows to
#                           M=32 so the full PSUM band is written)
WBLOB_F = 304
# SIM_SAFE: write full 32-row PSUM bands for sigma/rgb so CoreSim's
# uninitialized-read checker passes; on HW the junk rows are never read and
# narrow loads (M=1 / M=3) are ~25us faster.
SIM_SAFE = False
# sblob per group [16, 256]: w0aug lhsT [0:128), viewBaug lhsT [128:256)
SBLOB_F = 256
BANK = 512            # f32 elements per PSUM bank (per partition)


def _prep(x, model_indices, pts_w0, pts_b0, pts_w1, pts_b1,
          feat_w, feat_b, sigma_w, sigma_b, view_w, view_b, rgb_w, rgb_b):
    """Host-side routing + packing. Returns per-core device arrays and
    decode info."""
    x = np.asarray(x, np.float32)
    idx = np.asarray(model_indices).astype(np.int64)
    counts = np.bincount(idx, minlength=NUM_MODELS)

    expert_order = np.argsort(-counts, kind="stable")  # descending count
    caps = np.empty(NGROUPS, np.int64)
    for k in range(NGROUPS):
        win = expert_order[WIN * k:WIN * (k + 1)]
        c = int(counts[win].max())
        caps[k] = max(4, -(-c // 4) * 4)  # round up to multiple of 4, >=4
    assert caps.max() <= BANK
    colstart = np.concatenate([[0], np.cumsum(4 * caps)])
    w_tot = int(colstart[-1])

    order_pts = np.argsort(idx, kind="stable")
    starts = np.concatenate([[0], np.cumsum(counts)])

    # fold the feat layer into the view layer on the host:
    #   view(h) = relu(Wv [feat(h); views] + bv)
    #           = relu((Wv[:, :32] @ Wf) h + WvB views + (bv + Wv[:, :32] bf))
    vb_fold = view_b + np.einsum("goh,gh->go", view_w[:, :, :W], feat_b)
    vwA_fold = np.einsum("gox,gxh->goh", view_w[:, :, :W], feat_w)
    w0aug = np.concatenate(
        [np.transpose(pts_w0, (0, 2, 1)), pts_b0[:, None, :]], axis=1
    ).astype(np.float32)                      # [E, 4, 32] lhsT rows: xyz+bias
    vwBaug = np.concatenate(
        [np.transpose(view_w[:, :, W:], (0, 2, 1)), vb_fold[:, None, :]], axis=1
    ).astype(np.float32)                      # [E, 4, 32]
    w1T = np.transpose(pts_w1, (0, 2, 1)).astype(np.float32)    # [E,32,32]
    vwAT = np.transpose(vwA_fold, (0, 2, 1)).astype(np.float32)
    sigT = np.transpose(sigma_w, (0, 2, 1)).astype(np.float32)  # [E,32,1]
    rgbT = np.transpose(rgb_w, (0, 2, 1)).astype(np.float32)    # [E,32,3]
    b1 = np.asarray(pts_b1, np.float32)

    per_core = []
    decode = []  # (core, gid, pts, row_b, col_lo, cnt)
    for c in range(NCORES):
        gq = np.stack([expert_order[WIN * k + EPG * c: WIN * k + EPG * (c + 1)]
                       for k in range(NGROUPS)])  # [32, 16]

        wblob = np.zeros((NGROUPS, 128, WBLOB_F), np.float32)
        # B-step matrices (input layout B): l = 4b+s -> [k,(b kin),(s kout)]
        arr = w1T[gq].reshape(NGROUPS, 4, 4, W, W)    # [k,b,s,kin,kout]
        wblob[:, :, 0:128] = arr.transpose(0, 1, 3, 2, 4).reshape(NGROUPS, 128, 128)
        arr = vwAT[gq].reshape(NGROUPS, 4, 4, W, W)  # [k,j,i,kin,kout] (A)
        wblob[:, :, 128:256] = arr.transpose(0, 2, 3, 1, 4).reshape(NGROUPS, 128, 128)
        # A-step matrices (input layout A): l = 4j+i -> [k,(i kin),(j kout)]
        arr = sigT[gq].reshape(NGROUPS, 4, 4, W, 1)
        wblob[:, :, 256:260] = arr.transpose(0, 2, 3, 1, 4).reshape(NGROUPS, 128, 4)
        arr = rgbT[gq].reshape(NGROUPS, 4, 4, W, 3)   # [k,b,s,kin,kout] (B)
        wblob[:, :, 260:272] = arr.transpose(0, 1, 3, 2, 4).reshape(NGROUPS, 128, 12)
        # b1: L1 out layout A: partition 32s+h (s=e%4), slot q=e//4 -> col q
        arr = b1[gq].reshape(NGROUPS, 4, 4, W)        # [k,q,s,h]
        wblob[:, :, 272:276] = arr.transpose(0, 2, 3, 1).reshape(NGROUPS, 128, 4)

        sblob = np.zeros((NGROUPS, 16, SBLOB_F), np.float32)
        arr = w0aug[gq].reshape(NGROUPS, 4, 4, 4, W)  # [k,j,i,kin,kout] (A)
        sblob[:, :, 0:128] = arr.transpose(0, 2, 3, 1, 4).reshape(NGROUPS, 16, 128)
        arr = vwBaug[gq].reshape(NGROUPS, 4, 4, 4, W)  # [k,j,i,kin,kout] (A)
        sblob[:, :, 128:256] = arr.transpose(0, 2, 3, 1, 4).reshape(NGROUPS, 16, 128)

        xpts = np.zeros((16, w_tot), np.float32)
        views = np.zeros((16, w_tot), np.float32)
        xpts[3::4, :] = 1.0   # constant-1 rows for bias-in-matmul
        views[3::4, :] = 1.0
        for k in range(NGROUPS):
            C = int(caps[k])
            for l in range(EPG):
                gid = int(gq[k, l])
                cnt = int(counts[gid])
                pts = order_pts[starts[gid]:starts[gid] + cnt]
                i, j = l % 4, l // 4      # layout A (band, seg)
                ca = int(colstart[k]) + C * j   # A-seg columns
                cs = int(colstart[k]) + C * i   # B-seg columns
                if cnt:
                    xv = x[pts]
                    xpts[4 * i:4 * i + 3, ca:ca + cnt] = xv[:, :3].T
                    views[4 * i:4 * i + 3, ca:ca + cnt] = xv[:, 3:6].T
                # rgb lands at rows 4i+0..2, cols ca; sigma at row 4j+3, cols cs
                decode.append((c, gid, pts, i, j, ca, cs, cnt))
        per_core.append(dict(
            xpts=xpts, views=views,
            wblob=wblob.transpose(1, 0, 2).reshape(128, NGROUPS * WBLOB_F),
            sblob=sblob.transpose(1, 0, 2).reshape(16, NGROUPS * SBLOB_F)))

    return per_core, decode, caps, colstart, w_tot


def _build_nc(caps, w_tot):
    import concourse.mybir as mybir
    import concourse.tile as tile
    from concourse import bacc
    from contextlib import ExitStack

    f32 = mybir.dt.float32
    RELU = mybir.ActivationFunctionType.Relu
    ADD = mybir.AluOpType.add
    MAX = mybir.AluOpType.max

    nc = bacc.Bacc("TRN2", target_bir_lowering=False)
    xpts_d = nc.declare_dram_parameter("xpts", [16, w_tot], f32, isOutput=False)
    views_d = nc.declare_dram_parameter("views", [16, w_tot], f32, isOutput=False)
    wblob_d = nc.declare_dram_parameter("wblob", [128, NGROUPS * WBLOB_F], f32,
                                        isOutput=False)
    sblob_d = nc.declare_dram_parameter("sblob", [16, NGROUPS * SBLOB_F], f32,
                                        isOutput=False)
    out_d = nc.declare_dram_parameter("out", [16, w_tot], f32, isOutput=True)

    with tile.TileContext(nc) as tc, ExitStack() as ctx:
        const = ctx.enter_context(tc.tile_pool(name="const", bufs=1))
        hpool = ctx.enter_context(tc.tile_pool(name="h", bufs=8))
        pspool = ctx.enter_context(tc.tile_pool(name="ps", bufs=1, space="PSUM"))
        # One persistent 8-bank PSUM tensor, hand-slotted: a step claims
        # (bank-set, column-offset) slot; bank q within the set = row group q
        # (different row groups must not share a bank; same row group may).
        psall = pspool.tile([128, 8 * BANK], f32, tag="psall")
        step_ctr = [0]

        def ps_step():
            sidx = step_ctr[0]
            step_ctr[0] += 1
            bs = sidx % 2
            co = ((sidx // 2) % 8) * 64

            def mm_out(part_lo, m, q, C):
                base = (4 * bs + q) * BANK + co
                return psall[part_lo:part_lo + m, base:base + C]

            def copy_src(C):
                return psall.rearrange("p (b w) -> p b w", b=8)[
                    :, 4 * bs:4 * bs + 4, co:co + C]

            return mm_out, copy_src

        xt = const.tile([128, w_tot], f32)
        vt = const.tile([128, w_tot], f32)
        for i in range(4):
            nc.sync.dma_start(out=xt[32 * i:32 * i + 4, :],
                              in_=xpts_d[4 * i:4 * i + 4, :])
            nc.sync.dma_start(out=vt[32 * i:32 * i + 4, :],
                              in_=views_d[4 * i:4 * i + 4, :])
        wt_all = const.tile([128, NGROUPS * WBLOB_F], f32)
        wtot = NGROUPS * WBLOB_F
        nchunk = 8
        csz = -(-wtot // nchunk)
        for u in range(nchunk):
            lo, hi = u * csz, min((u + 1) * csz, wtot)
            nc.sync.dma_start(out=wt_all[:, lo:hi], in_=wblob_d[:, lo:hi])
        st_all = const.tile([128, NGROUPS * SBLOB_F], f32)
        for i in range(4):
            nc.sync.dma_start(out=st_all[32 * i:32 * i + 4, :],
                              in_=sblob_d[4 * i:4 * i + 4, :])
        otr_all = const.tile([128, w_tot], f32)
        ots_all = const.tile([128, w_tot], f32)

        # Software-pipeline: emit steps step-major over windows of PIPE
        # groups so the PE always has another group's matmuls to run while
        # a step's PSUM->SBUF copy completes.
        PIPE = 4
        colstarts = np.concatenate([[0], np.cumsum(4 * np.asarray(caps))])

        def group_steps(g):
            C = int(caps[g])
            W4 = 4 * C
            col = int(colstarts[g])
            wt = wt_all[:, g * WBLOB_F:(g + 1) * WBLOB_F]
            st = st_all[:, g * SBLOB_F:(g + 1) * SBLOB_F]
            state = {}

            def s_l0():
                mm0, cp0 = ps_step()
                for j in range(4):
                    for i in range(4):
                        nc.tensor.matmul(
                            out=mm0(32 * j, 32, i, C),
                            lhsT=st[32 * i:32 * i + 4, 32 * j:32 * j + 32],
                            rhs=xt[32 * i:32 * i + 4, col + C * j:col + C * j + C],
                            start=True, stop=True, skip_group_check=True,
                            tile_position=(32 * i, 32 * j))
                h1 = hpool.tile([128, W4], f32, tag="h1")
                nc.scalar.activation(h1.rearrange("p (q w) -> p q w", q=4),
                                     cp0(C), RELU)
                state["h1"] = h1

            def s_l1():
                h1 = state.pop("h1")
                mm1, _ = ps_step()
                for s in range(4):
                    for b in range(4):
                        nc.tensor.matmul(
                            out=mm1(32 * s, 32, b, C),
                            lhsT=wt[32 * b:32 * b + 32, 32 * s:32 * s + 32],
                            rhs=h1[32 * b:32 * b + 32, C * s:C * s + C],
                            start=True, stop=True, skip_group_check=True,
                            tile_position=(32 * b, 32 * s))
                h2 = hpool.tile([128, W4], f32, tag="h2")
                for q in range(4):
                    nc.vector.tensor_scalar(
                        out=h2[:, C * q:C * q + C],
                        in0=mm1(0, 128, q, C),
                        scalar1=wt[:, 272 + q:273 + q], scalar2=0.0,
                        op0=ADD, op1=MAX)
                state["h2"] = h2

            def s_sigma():
                h2 = state["h2"]
                MS = 32 if SIM_SAFE else 1
                mms_, cps = ps_step()
                for j in range(4):
                    for i in range(4):
                        nc.tensor.matmul(
                            out=mms_(32 * j, MS, i, C),
                            lhsT=wt[32 * i:32 * i + 32, 256 + j:256 + j + MS],
                            rhs=h2[32 * i:32 * i + 32, C * j:C * j + C],
                            start=True, stop=True, skip_group_check=True,
                            tile_position=(32 * i, 32 * j))
                nc.scalar.copy(
                    ots_all[:, col:col + W4].rearrange("p (q w) -> p q w", q=4),
                    cps(C))

            def s_view():
                h2 = state.pop("h2")
                mmv, cpv = ps_step()
                for j in range(4):
                    for i in range(4):
                        nc.tensor.matmul(
                            out=mmv(32 * j, 32, i, C),
                            lhsT=wt[32 * i:32 * i + 32, 128 + 32 * j:128 + 32 * j + 32],
                            rhs=h2[32 * i:32 * i + 32, C * j:C * j + C],
                            start=True, stop=False, skip_group_check=True,
                            tile_position=(32 * i, 32 * j))
                    for i in range(4):
                        nc.tensor.matmul(
                            out=mmv(32 * j, 32, i, C),
                            lhsT=st[32 * i:32 * i + 4, 128 + 32 * j:128 + 32 * j + 32],
                            rhs=vt[32 * i:32 * i + 4, col + C * j:col + C * j + C],
                            start=False, stop=True, skip_group_check=True,
                            tile_position=(32 * i, 32 * j))
                hv = hpool.tile([128, W4], f32, tag="hv")
                nc.scalar.activation(hv.rearrange("p (q w) -> p q w", q=4),
                                     cpv(C), RELU)
                state["hv"] = hv

            def s_rgb():
                hv = state.pop("hv")
                MR = 32 if SIM_SAFE else 3
                mmr, cpr = ps_step()
                for s in range(4):
                    for b in range(4):
                        nc.tensor.matmul(
                            out=mmr(32 * s, MR, b, C),
                            lhsT=wt[32 * b:32 * b + 32, 260 + 3 * s:260 + 3 * s + MR],
                            rhs=hv[32 * b:32 * b + 32, C * s:C * s + C],
                            start=True, stop=True, skip_group_check=True,
                            tile_position=(32 * b, 32 * s))
                nc.vector.tensor_copy(
                    otr_all[:, col:col + W4].rearrange("p (q w) -> p q w", q=4),
                    cpr(C))

            return [s_l0, s_l1, s_sigma, s_view, s_rgb]

        for base in range(0, NGROUPS, PIPE):
            window = [group_steps(g)
                      for g in range(base, min(base + PIPE, NGROUPS))]
            for stepi in range(5):
                for steps in window:
                    steps[stepi]()

        for b in range(4):
            nc.sync.dma_start(out=out_d[4 * b:4 * b + 3, :],
                              in_=otr_all[32 * b:32 * b + 3, :])
            nc.sync.dma_start(out=out_d[4 * b + 3:4 * b + 4, :],
                              in_=ots_all[32 * b:32 * b + 1, :])

    nc.compile()
    return nc


def _decode_out(results, decode, sigma_b, rgb_b):
    y = np.empty((N, 4), np.float32)
    outs = [np.asarray(r["out"]) for r in results]
    for (c, gid, pts, i, j, ca, cs, cnt) in decode:
        if cnt == 0:
            continue
        o = outs[c]
        y[pts, 0:3] = o[4 * i:4 * i + 3, ca:ca + cnt].T + rgb_b[gid]
        y[pts, 3] = o[4 * j + 3, cs:cs + cnt] + sigma_b[gid, 0]
    return y


def kernel(**inputs):
    from concourse.bass_utils import run_bass_kernel_spmd

    per_core, decode, caps, colstart, w_tot = _prep(**inputs)
    nc = _build_nc(caps, w_tot)
    in_maps = [per_core[c] for c in range(NCORES)]
    res = run_bass_kernel_spmd(nc, in_maps, list(range(NCORES)))
    return _decode_out(res.results, decode,
                       np.asarray(inputs["sigma_b"], np.float32),
                       np.asarray(inputs["rgb_b"], np.float32))


# ---------------------------------------------------------------------------
# numpy emulation of the device program (for layout validation in test.py)
def _emulate_core(arrs, caps, w_tot):
    xt = np.zeros((128, w_tot), np.float32)
    vt = np.zeros((128, w_tot), np.float32)
    for i in range(4):
        xt[32 * i:32 * i + 4] = arrs["xpts"][4 * i:4 * i + 4]
        vt[32 * i:32 * i + 4] = arrs["views"][4 * i:4 * i + 4]
    out = np.zeros((16, w_tot), np.float32)
    col = 0
    for g in range(NGROUPS):
        C = int(caps[g])
        W4 = 4 * C
        wt = arrs["wblob"][:, g * WBLOB_F:(g + 1) * WBLOB_F]
        st = np.zeros((128, SBLOB_F), np.float32)
        for i in range(4):
            st[32 * i:32 * i + 4] = arrs["sblob"][4 * i:4 * i + 4,
                                                  g * SBLOB_F:(g + 1) * SBLOB_F]

        ps0 = np.zeros((128, W4), np.float32)
        for l in range(EPG):
            i, j = l % 4, l // 4
            ps0[32 * j:32 * j + 32, C * i:C * i + C] = (
                st[32 * i:32 * i + 4, 32 * j:32 * j + 32].T
                @ xt[32 * i:32 * i + 4, col + C * j:col + C * j + C])
        h1 = np.maximum(ps0, 0)
        ps1 = np.zeros((128, W4), np.float32)
        for l in range(EPG):
            b, s = l // 4, l % 4
            ps1[32 * s:32 * s + 32, C * b:C * b + C] = (
                wt[32 * b:32 * b + 32, 32 * s:32 * s + 32].T
                @ h1[32 * b:32 * b + 32, C * s:C * s + C])
        h2 = np.empty_like(ps1)
        for q in range(4):
            h2[:, C * q:C * q + C] = np.maximum(
                ps1[:, C * q:C * q + C] + wt[:, 272 + q:273 + q], 0)
        pss = np.zeros((128, W4), np.float32)
        for l in range(EPG):
            i, j = l % 4, l // 4
            rhs = h2[32 * i:32 * i + 32, C * j:C * j + C]
            pss[32 * j:32 * j + 1, C * i:C * i + C] = (
                wt[32 * i:32 * i + 32, 256 + j:257 + j].T @ rhs)
        psv = np.zeros((128, W4), np.float32)
        for l in range(EPG):
            i, j = l % 4, l // 4
            psv[32 * j:32 * j + 32, C * i:C * i + C] = (
                wt[32 * i:32 * i + 32, 128 + 32 * j:128 + 32 * j + 32].T
                @ h2[32 * i:32 * i + 32, C * j:C * j + C]
                + st[32 * i:32 * i + 4, 128 + 32 * j:128 + 32 * j + 32].T
                @ vt[32 * i:32 * i + 4, col + C * j:col + C * j + C])
        hv = np.maximum(psv, 0)
        psr = np.zeros((128, W4), np.float32)
        for l in range(EPG):
            b, s = l // 4, l % 4
            psr[32 * s:32 * s + 3, C * b:C * b + C] = (
                wt[32 * b:32 * b + 32, 260 + 3 * s:263 + 3 * s].T
                @ hv[32 * b:32 * b + 32, C * s:C * s + C])
        for b in range(4):
            out[4 * b:4 * b + 3, col:col + W4] = psr[32 * b:32 * b + 3, :]
            out[4 * b + 3, col:col + W4] = pss[32 * b, :]
        col += W4
    return out


def kernel_emulated(**inputs):
    per_core, decode, caps, colstart, w_tot = _prep(**inputs)
    results = [{"out": _emulate_core(per_core[c], caps, w_tot)}
               for c in range(NCORES)]
    return _decode_out(results, decode,
                       np.asarray(inputs["sigma_b"], np.float32),
                       np.asarray(inputs["rgb_b"], np.float32))



# revision 2
# speedup vs baseline: 1.2051x; 1.2051x over previous
"""BatchedKiloNeRF Trainium2 kernel, v2: block-diagonal expert packing.

Strategy:
  - 4096 experts sorted by point count; windows of 32 consecutive experts
    give each of the 8 cores one stack of 4 experts with a SHARED capacity
    C (max count in window, rounded to 4) so the SPMD program is identical
    across cores. 128 stacks per core.
  - A stack of 4 experts runs each layer as ONE full-array matmul with a
    block-diagonal lhsT [128,128] (expert i occupies rows/cols 32i:32i+32).
    Off-diagonal zeros guarantee band i of the output depends only on band
    i of the input, so per-expert chains never mix. bf16 everywhere on the
    PE; PSUM accumulates f32.
  - Points of expert i live in partition band 32i of the hidden tiles, at
    the stack's column window (width C). Layer chain per stack:
      l0   [16,128] lhsT (coords+const-1 row per expert -> bias in matmul)
      l1   [128,128] block-diag; b1 bias pre-filled into PSUM by ONE
           indicator matmul per super-group (lhsT = bias table [S,128],
           rhs = 0/1 stack-indicator tile), l1 accumulates on top.
      sigma [128,4] lhsT -> psD rows 0:4 (row i = expert i sigma)
      view  [128,128] block-diag + [16,128] views part (bias via const-1)
      rgb  [128,12] lhsT -> psD rows 4:16 (rows 4+3i:4+3i+3 = expert i)
  - Super-groups (SG): consecutive stacks, sum C <= 512 (one PSUM bank),
    <= 16 stacks. Per SG one batched ACT/DVE relu per layer boundary.
    PSUM: role (l0,l1,view,out) x SG parity = 8 banks.
  - Weights are DMAed as pre-padded bf16 blobs (zeros baked in DRAM).
"""

import sys

import numpy as np
import ml_dtypes

for _p in ("/opt/trn_rl_repo",):
    if _p not in sys.path:
        sys.path.append(_p)

NUM_MODELS = 4096
W = 32
N = 131072
NCORES = 8
NSTACK = 128          # stacks per core (windows of 32 experts globally)
BANK = 512
SGMAXC = 512          # columns per super-group (one PSUM bank)
SGMAXS = 16           # stacks per super-group (indicator tile partitions)
WB128F = 144          # per-stack cols in wb128: l1 64 | viewA 64 | sig 4 | rgb 12
WB16F = 256           # per-stack cols in wb16: l0aug 128 | viewBaug 128
PIPE = 2              # SG emission interleave (must divide bank parity)

BF16 = ml_dtypes.bfloat16


def _plan(counts):
    """Global (core-independent) packing plan. Returns expert order, caps,
    colstart, w_tot, super-groups."""
    order = np.argsort(-counts, kind="stable")
    win = order.reshape(NSTACK, 32)                      # window k -> 32 experts
    caps = np.maximum(4, -(-counts[win].max(axis=1) // 4) * 4)
    assert caps.max() <= BANK
    colstart = np.concatenate([[0], np.cumsum(caps)]).astype(np.int64)
    w_tot = int(colstart[-1])
    sgs = []
    s0 = 0
    while s0 < NSTACK:
        s1 = s0
        tot = 0
        while s1 < NSTACK and s1 - s0 < SGMAXS and tot + caps[s1] <= SGMAXC:
            tot += caps[s1]
            s1 += 1
        sgs.append((s0, s1))
        s0 = s1
    # stack pairs within each SG for K-stacked l0/viewB matmuls
    pairs = []
    pair_of = {}
    for (s0_, s1_) in sgs:
        k = s0_
        while k < s1_:
            k2 = k + 1 if k + 1 < s1_ else -1
            pair_of[k] = (len(pairs), 0)
            if k2 >= 0:
                pair_of[k2] = (len(pairs), 1)
            pairs.append((k, k2))
            k += 2
    return order, win, caps, colstart, w_tot, sgs, pairs, pair_of


def _prep(x, model_indices, pts_w0, pts_b0, pts_w1, pts_b1,
          feat_w, feat_b, sigma_w, sigma_b, view_w, view_b, rgb_w, rgb_b):
    x = np.asarray(x, np.float32)
    idx = np.asarray(model_indices).astype(np.int64)
    counts = np.bincount(idx, minlength=NUM_MODELS)
    order, win, caps, colstart, w_tot, sgs, pairs, pair_of = _plan(counts)
    nsg = len(sgs)

    order_pts = np.argsort(idx, kind="stable")
    starts = np.concatenate([[0], np.cumsum(counts)])

    pts_w0 = np.asarray(pts_w0, np.float32)
    pts_b0 = np.asarray(pts_b0, np.float32)
    pts_w1 = np.asarray(pts_w1, np.float32)
    pts_b1 = np.asarray(pts_b1, np.float32)
    feat_w = np.asarray(feat_w, np.float32)
    feat_b = np.asarray(feat_b, np.float32)
    sigma_w = np.asarray(sigma_w, np.float32)
    view_w = np.asarray(view_w, np.float32)
    rgb_w = np.asarray(rgb_w, np.float32)

    # fold feat layer into view layer (baseline trick):
    # view(h) = relu(Wv[:, :32] Wf h + Wv[:, 32:] views + bv + Wv[:, :32] bf)
    vwA = np.einsum("gox,gxh->goh", view_w[:, :, :W], feat_w)   # [E, o, h]
    vb_fold = view_b + np.einsum("goh,gh->go", view_w[:, :, :W], feat_b)

    w0aug = np.concatenate(
        [np.transpose(pts_w0, (0, 2, 1)), pts_b0[:, None, :]], axis=1
    )                                                   # [E, 4(kin), 32]
    vwBaug = np.concatenate(
        [np.transpose(view_w[:, :, W:], (0, 2, 1)), vb_fold[:, None, :]], axis=1
    )                                                   # [E, 4(kin), 32]
    w1T = np.transpose(pts_w1, (0, 2, 1))               # [E, a(in), h(out)]
    vwAT = np.transpose(vwA, (0, 2, 1))                 # [E, h(in), o(out)]

    # map stack index -> sg index and local row
    sg_of = np.empty(NSTACK, np.int64)
    loc_of = np.empty(NSTACK, np.int64)
    for g, (s0, s1) in enumerate(sgs):
        sg_of[s0:s1] = g
        loc_of[s0:s1] = np.arange(s1 - s0)

    per_core = []
    decode = []   # (core, stack, i, expert, cnt, col)
    for c in range(NCORES):
        gq = win[:, 4 * c:4 * c + 4]                    # [128, 4]

        xpts = np.zeros((32, w_tot), np.float32)
        views = np.zeros((32, w_tot), np.float32)
        ind = np.zeros((16, w_tot), np.float32)
        b1tab = np.zeros((16, 128 * nsg), np.float32)
        wb128 = np.zeros((128, WB128F * NSTACK), np.float32)
        wb16 = np.zeros((32, WB16F * len(pairs)), np.float32)

        for k in range(NSTACK):
            col = int(colstart[k])
            C = int(caps[k])
            g, loc = int(sg_of[k]), int(loc_of[k])
            ind[loc, col:col + C] = 1.0
            b1tab[loc, 128 * g:128 * (g + 1)] = pts_b1[gq[k]].reshape(128)
            w0 = WB128F * k
            pi, half = pair_of[k]
            s0 = WB16F * pi
            hb = 16 * half
            for i in range(4):
                e = int(gq[k, i])
                r = slice(32 * i, 32 * i + 32)
                # [64,64] half-blocks: expert i sits at rows 32i, block col
                # 32*(i%2) of half i//2; halves share the same 64 columns
                bc = 32 * (i % 2)
                wb128[r, w0 + bc:w0 + bc + 32] = w1T[e]
                wb128[r, w0 + 64 + bc:w0 + 96 + bc] = vwAT[e]
                wb128[r, w0 + 128 + i] = sigma_w[e, 0]
                wb128[r, w0 + 132 + 3 * i:w0 + 135 + 3 * i] = rgb_w[e].T
                wb16[hb + 4 * i:hb + 4 * i + 4,
                     s0 + 32 * i:s0 + 32 * i + 32] = w0aug[e]
                wb16[hb + 4 * i:hb + 4 * i + 4,
                     s0 + 128 + 32 * i:s0 + 160 + 32 * i] = vwBaug[e]
                cnt = int(counts[e])
                pts = order_pts[starts[e]:starts[e] + cnt]
                if cnt:
                    xv = x[pts]
                    xpts[hb + 4 * i:hb + 4 * i + 3, col:col + cnt] = xv[:, :3].T
                    views[hb + 4 * i:hb + 4 * i + 3, col:col + cnt] = \
                        xv[:, 3:6].T
                xpts[hb + 4 * i + 3, col:col + C] = 1.0
                views[hb + 4 * i + 3, col:col + C] = 1.0
                decode.append((c, k, i, e, cnt, col))

        per_core.append(dict(
            xpts=xpts.astype(BF16), views=views.astype(BF16),
            ind=ind.astype(BF16), b1tab=b1tab.astype(BF16),
            wb128=wb128.astype(BF16), wb16=wb16.astype(BF16)))

    return (per_core, decode, order_pts, starts, caps, colstart,
            w_tot, sgs, pairs, pair_of)


def _build_nc(caps, colstart, w_tot, sgs, pairs, pair_of):
    import concourse.mybir as mybir
    import concourse.tile as tile
    from concourse import bacc
    from contextlib import ExitStack

    f32 = mybir.dt.float32
    bf16 = mybir.dt.bfloat16
    RELU = mybir.ActivationFunctionType.Relu
    nsg = len(sgs)

    nc = bacc.Bacc("TRN2", target_bir_lowering=False)
    xpts_d = nc.declare_dram_parameter("xpts", [32, w_tot], bf16, isOutput=False)
    views_d = nc.declare_dram_parameter("views", [32, w_tot], bf16, isOutput=False)
    ind_d = nc.declare_dram_parameter("ind", [16, w_tot], bf16, isOutput=False)
    b1_d = nc.declare_dram_parameter("b1tab", [16, 128 * nsg], bf16, isOutput=False)
    wb128_d = nc.declare_dram_parameter("wb128", [128, WB128F * NSTACK], bf16,
                                        isOutput=False)
    wb16_d = nc.declare_dram_parameter("wb16", [32, WB16F * len(pairs)], bf16,
                                       isOutput=False)
    out_d = nc.declare_dram_parameter("out", [16, w_tot], f32, isOutput=True)

    with tile.TileContext(nc) as tc, ExitStack() as ctx:
        const = ctx.enter_context(tc.tile_pool(name="const", bufs=1))
        hpool = ctx.enter_context(tc.tile_pool(name="h", bufs=6))
        pspool = ctx.enter_context(tc.tile_pool(name="ps", bufs=1, space="PSUM"))

        xt = const.tile([32, w_tot], bf16)
        vt = const.tile([32, w_tot], bf16)
        it = const.tile([16, w_tot], bf16)
        bt = const.tile([16, 128 * nsg], bf16)
        wt128 = const.tile([128, WB128F * NSTACK], bf16)
        wt16 = const.tile([32, WB16F * len(pairs)], bf16)
        so = const.tile([44, w_tot], f32)
        ps = pspool.tile([128, 8 * BANK], f32, tag="ps")

        # Input DMAs: all issued up front (no deps), in consumption order,
        # alternating the SP and ACT HWDGE rings so transfers run on two
        # queues in parallel. Small first chunks let compute start early.
        # Both sequencers are otherwise idle at kernel start, so the issue
        # cost never delays compute ops.
        chunks = [(0, 1), (1, 2), (2, 3), (3, 4), (4, 6), (6, 8), (8, nsg)]
        chunks = [(a, min(b, nsg)) for a, b in chunks if a < min(b, nsg)]
        qs = [nc.sync, nc.scalar]
        qi = 0

        def pair_range(g0, g1):
            s0, s1 = sgs[g0][0], sgs[g1 - 1][1]
            return pair_of[s0][0], pair_of[s1 - 1][0] + 1

        # the very first l0 matmul needs only SG0's xt + wt16 slices; put
        # those tiny DMAs first so the pipeline starts right after the
        # framework preamble instead of behind the first weight chunk
        fs0, fs1 = sgs[0]
        flo, fhi = int(colstart[fs0]), int(colstart[fs1])
        fp0, fp1 = pair_range(0, 1)
        nc.sync.dma_start(out=xt[:, flo:fhi], in_=xpts_d[:, flo:fhi])
        nc.scalar.dma_start(out=wt16[:, WB16F * fp0:WB16F * fp1],
                            in_=wb16_d[:, WB16F * fp0:WB16F * fp1])
        nc.gpsimd.dma_start(out=bt, in_=b1_d[:, :])
        for g0, g1 in chunks:
            s0, s1 = sgs[g0][0], sgs[g1 - 1][1]
            lo, hi = int(colstart[s0]), int(colstart[s1])
            p0, p1 = pair_range(g0, g1)
            first = g0 == 0
            # wb128 (9.4MB) split: ~45% sync, ~45% scalar, ~10% gpsimd so
            # all three rings (~112/112/85 GB/s) finish together with the
            # small tensors also on the gpsimd SWDGE ring.
            m1 = s0 + max(1, round((s1 - s0) * 0.45))
            m2 = min(s1, m1 + max(1, round((s1 - s0) * 0.45)))
            qa, qb = qs[qi % 2], qs[(qi + 1) % 2]
            qi += 1
            qa.dma_start(out=wt128[:, WB128F * s0:WB128F * m1],
                         in_=wb128_d[:, WB128F * s0:WB128F * m1])
            qb.dma_start(out=wt128[:, WB128F * m1:WB128F * m2],
                         in_=wb128_d[:, WB128F * m1:WB128F * m2])
            if m2 < s1:
                nc.gpsimd.dma_start(out=wt128[:, WB128F * m2:WB128F * s1],
                                    in_=wb128_d[:, WB128F * m2:WB128F * s1])
            if not first:
                nc.gpsimd.dma_start(out=wt16[:, WB16F * p0:WB16F * p1],
                                    in_=wb16_d[:, WB16F * p0:WB16F * p1])
                nc.gpsimd.dma_start(out=xt[:, lo:hi], in_=xpts_d[:, lo:hi])
            nc.gpsimd.dma_start(out=vt[:, lo:hi], in_=views_d[:, lo:hi])
            nc.gpsimd.dma_start(out=it[:, lo:hi], in_=ind_d[:, lo:hi])

        def sg_steps(g):
            s0, s1 = sgs[g]
            Ssg = s1 - s0
            lo = int(colstart[s0])
            sgw = int(colstart[s1]) - lo
            # Role-shared banks: l0/view share a bank (disjoint lifetimes:
            # l0->relu1 then view->relu3), as do l1 and sigma/rgb. 2 banks
            # per SG x 4-deep parity = 8 banks, a 4-SG pipeline.
            p = g % 4
            psA = ps[:, p * BANK:p * BANK + sgw]
            psB = ps[:, (4 + p) * BANK:(4 + p) * BANK + sgw]
            psC = psA
            psD = psB
            state = {}

            def cw(k):
                a = int(colstart[k]) - lo
                return a, a + int(caps[k])

            sg_pairs = [pairs[pi] for pi in
                        range(pair_of[s0][0], pair_of[s1 - 1][0] + 1)]

            def pw(pr):
                k, k2 = pr
                a = int(colstart[k]) - lo
                b = (int(colstart[k2]) + int(caps[k2]) - lo
                     if k2 >= 0 else a + int(caps[k]))
                return a, b

            def s_l0():
                for n, pr in enumerate(sg_pairs):
                    pi = pair_of[pr[0]][0]
                    a, b = pw(pr)
                    nc.tensor.matmul(
                        out=psA[:, a:b],
                        lhsT=wt16[:, WB16F * pi:WB16F * pi + 128],
                        rhs=xt[:, lo + a:lo + b],
                        start=(n == 0), stop=(n == len(sg_pairs) - 1),
                        skip_group_check=True)

            def s_relu1():
                h1 = hpool.tile([128, SGMAXC], bf16, tag="h1")
                nc.scalar.activation(h1[:, 0:sgw], psA, RELU)
                state["h1"] = h1

            def s_l1():
                h1 = state.pop("h1")
                nc.tensor.matmul(
                    out=psB,
                    lhsT=bt[0:Ssg, 128 * g:128 * (g + 1)],
                    rhs=it[0:Ssg, lo:lo + sgw],
                    start=True, stop=False, skip_group_check=True)
                for k in range(s0, s1):
                    a, b = cw(k)
                    for hf in range(2):
                        nc.tensor.matmul(
                            out=psB[64 * hf:64 * hf + 64, a:b],
                            lhsT=wt128[64 * hf:64 * hf + 64,
                                       WB128F * k:WB128F * k + 64],
                            rhs=h1[64 * hf:64 * hf + 64, a:b],
                            start=False, stop=(k == s1 - 1 and hf == 1),
                            skip_group_check=True,
                            tile_position=(64 * hf, 64 * hf))

            def s_relu2():
                h2 = hpool.tile([128, SGMAXC], bf16, tag="h2")
                nc.vector.tensor_scalar_max(h2[:, 0:sgw], psB, 0.0)
                state["h2"] = h2

            def s_sigview():
                h2 = state.pop("h2")
                for k in range(s0, s1):
                    a, b = cw(k)
                    nc.tensor.matmul(
                        out=psD[0:4, a:b],
                        lhsT=wt128[:, WB128F * k + 128:WB128F * k + 132],
                        rhs=h2[:, a:b],
                        start=(k == s0), stop=(k == s1 - 1),
                        skip_group_check=True)
                for k in range(s0, s1):
                    a, b = cw(k)
                    for hf in range(2):
                        nc.tensor.matmul(
                            out=psC[64 * hf:64 * hf + 64, a:b],
                            lhsT=wt128[64 * hf:64 * hf + 64,
                                       WB128F * k + 64:WB128F * k + 128],
                            rhs=h2[64 * hf:64 * hf + 64, a:b],
                            start=(k == s0), stop=False,
                            skip_group_check=True,
                            tile_position=(64 * hf, 64 * hf))
                for n, pr in enumerate(sg_pairs):
                    pi = pair_of[pr[0]][0]
                    a, b = pw(pr)
                    nc.tensor.matmul(
                        out=psC[:, a:b],
                        lhsT=wt16[:, WB16F * pi + 128:WB16F * pi + 256],
                        rhs=vt[:, lo + a:lo + b],
                        start=False, stop=(n == len(sg_pairs) - 1),
                        skip_group_check=True)

            def s_relu3():
                hv = hpool.tile([128, SGMAXC], bf16, tag="hv")
                nc.scalar.activation(hv[:, 0:sgw], psC, RELU)
                state["hv"] = hv

            def s_rgb():
                hv = state.pop("hv")
                for k in range(s0, s1):
                    a, b = cw(k)
                    nc.tensor.matmul(
                        out=psD[32:44, a:b],
                        lhsT=wt128[:, WB128F * k + 132:WB128F * k + 144],
                        rhs=hv[:, a:b],
                        start=(k == s0), stop=(k == s1 - 1),
                        skip_group_check=True)

            def s_out():
                nc.vector.tensor_copy(so[32:44, lo:lo + sgw], psD[32:44, :])
                nc.vector.tensor_copy(so[0:4, lo:lo + sgw], psD[0:4, :])
                nc.sync.dma_start(out=out_d[0:12, lo:lo + sgw],
                                  in_=so[32:44, lo:lo + sgw])
                nc.sync.dma_start(out=out_d[12:16, lo:lo + sgw],
                                  in_=so[0:4, lo:lo + sgw])

            return [s_l0, s_relu1, s_l1, s_relu2, s_sigview, s_relu3,
                    s_rgb, s_out]

        # Skewed modulo schedule: at tick t the PE runs l0(t), l1(t-1),
        # sigview(t-2), rgb(t-3), so each step's relu finished a full tick
        # (~3us of PE work) before its consumer issues. PSUM parity-2
        # lifetimes are exactly 2 ticks, matching the bank assignment.
        steps = [sg_steps(g) for g in range(nsg)]
        skew = [0, 0, 1, 1, 2, 2, 3, 3]
        for t in range(nsg + 3):
            for j in range(8):
                g = t - skew[j]
                if 0 <= g < nsg:
                    steps[g][j]()

    nc.compile()
    return nc


def _finish(results, decode, order_pts, starts, sigma_b, rgb_b):
    y = np.empty((N, 4), np.float32)
    outs = [np.asarray(r["out"]) for r in results]
    for (c, k, i, e, cnt, col) in decode:
        if cnt == 0:
            continue
        o = outs[c]
        pts = order_pts[starts[e]:starts[e] + cnt]
        y[pts, 0:3] = o[3 * i:3 * i + 3, col:col + cnt].T + rgb_b[e]
        y[pts, 3] = o[12 + i, col:col + cnt] + sigma_b[e, 0]
    return y


def kernel(**inputs):
    from concourse.bass_utils import run_bass_kernel_spmd

    (per_core, decode, order_pts, starts, caps, colstart, w_tot, sgs,
     pairs, pair_of) = _prep(**inputs)
    nc = _build_nc(caps, colstart, w_tot, sgs, pairs, pair_of)
    res = run_bass_kernel_spmd(nc, per_core, list(range(NCORES)))
    return _finish(res.results, decode, order_pts, starts,
                   np.asarray(inputs["sigma_b"], np.float32),
                   np.asarray(inputs["rgb_b"], np.float32))


# ---------------------------------------------------------------------------
# numpy emulation of the device program (layout validation)
def _emulate_core(arrs, caps, colstart, w_tot, sgs, pairs, pair_of):
    f = np.float32
    xt = arrs["xpts"].astype(f)
    vt = arrs["views"].astype(f)
    it = arrs["ind"].astype(f)
    bt = arrs["b1tab"].astype(f)
    wt128 = arrs["wb128"].astype(f)
    wt16 = arrs["wb16"].astype(f)
    out = np.zeros((16, w_tot), f)
    for g, (s0, s1) in enumerate(sgs):
        Ssg = s1 - s0
        lo = int(colstart[s0])
        sgw = int(colstart[s1]) - lo
        psA = np.zeros((128, sgw), f)
        psB = np.zeros((128, sgw), f)
        psC = np.zeros((128, sgw), f)
        psD = np.zeros((16, sgw), f)
        k = s0
        while k < s1:
            pi, _ = pair_of[k]
            kk, k2 = pairs[pi]
            a = int(colstart[kk]) - lo
            b = (int(colstart[k2]) + int(caps[k2]) - lo
                 if k2 >= 0 else a + int(caps[kk]))
            psA[:, a:b] = wt16[:, WB16F * pi:WB16F * pi + 128].T @ \
                xt[:, lo + a:lo + b]
            k = k2 + 1 if k2 >= 0 else k + 1
        h1 = np.maximum(psA, 0).astype(BF16).astype(f)
        psB[:] = bt[0:Ssg, 128 * g:128 * (g + 1)].T @ it[0:Ssg, lo:lo + sgw]
        for k in range(s0, s1):
            a = int(colstart[k]) - lo
            b = a + int(caps[k])
            for hf in range(2):
                r = slice(64 * hf, 64 * hf + 64)
                psB[r, a:b] += \
                    wt128[r, WB128F * k:WB128F * k + 64].T @ h1[r, a:b]
        h2 = np.maximum(psB, 0).astype(BF16).astype(f)
        for k in range(s0, s1):
            a = int(colstart[k]) - lo
            b = a + int(caps[k])
            psD[0:4, a:b] = \
                wt128[:, WB128F * k + 128:WB128F * k + 132].T @ h2[:, a:b]
            pi, _ = pair_of[k]
            psC[:, a:b] = wt16[:, WB16F * pi + 128:WB16F * pi + 256].T @ \
                vt[:, lo + a:lo + b]
            for hf in range(2):
                r = slice(64 * hf, 64 * hf + 64)
                psC[r, a:b] += \
                    wt128[r, WB128F * k + 64:WB128F * k + 128].T @ h2[r, a:b]
        hv = np.maximum(psC, 0).astype(BF16).astype(f)
        for k in range(s0, s1):
            a = int(colstart[k]) - lo
            b = a + int(caps[k])
            psD[4:16, a:b] = \
                wt128[:, WB128F * k + 132:WB128F * k + 144].T @ hv[:, a:b]
        out[0:12, lo:lo + sgw] = psD[4:16]
        out[12:16, lo:lo + sgw] = psD[0:4]
    return out


def kernel_emulated(**inputs):
    (per_core, decode, order_pts, starts, caps, colstart, w_tot, sgs,
     pairs, pair_of) = _prep(**inputs)
    results = [{"out": _emulate_core(per_core[c], caps, colstart, w_tot, sgs,
                                     pairs, pair_of)}
               for c in range(NCORES)]
    return _finish(results, decode, order_pts, starts,
                   np.asarray(inputs["sigma_b"], np.float32),
                   np.asarray(inputs["rgb_b"], np.float32))


# revision 3
# speedup vs baseline: 1.2455x; 1.0335x over previous
"""BatchedKiloNeRF Trainium2 kernel, v2: block-diagonal expert packing.

Strategy:
  - 4096 experts sorted by point count; windows of 32 consecutive experts
    give each of the 8 cores one stack of 4 experts with a SHARED capacity
    C (max count in window, rounded to 4) so the SPMD program is identical
    across cores. 128 stacks per core.
  - A stack of 4 experts runs each layer as ONE full-array matmul with a
    block-diagonal lhsT [128,128] (expert i occupies rows/cols 32i:32i+32).
    Off-diagonal zeros guarantee band i of the output depends only on band
    i of the input, so per-expert chains never mix. bf16 everywhere on the
    PE; PSUM accumulates f32.
  - Points of expert i live in partition band 32i of the hidden tiles, at
    the stack's column window (width C). Layer chain per stack:
      l0   [16,128] lhsT (coords+const-1 row per expert -> bias in matmul)
      l1   [128,128] block-diag; b1 bias pre-filled into PSUM by ONE
           indicator matmul per super-group (lhsT = bias table [S,128],
           rhs = 0/1 stack-indicator tile), l1 accumulates on top.
      sigma [128,4] lhsT -> psD rows 0:4 (row i = expert i sigma)
      view  [128,128] block-diag + [16,128] views part (bias via const-1)
      rgb  [128,12] lhsT -> psD rows 4:16 (rows 4+3i:4+3i+3 = expert i)
  - Super-groups (SG): consecutive stacks, sum C <= 512 (one PSUM bank),
    <= 16 stacks. Per SG one batched ACT/DVE relu per layer boundary.
    PSUM: role (l0,l1,view,out) x SG parity = 8 banks.
  - Weights are DMAed as pre-padded bf16 blobs (zeros baked in DRAM).
"""

import sys

import numpy as np
import ml_dtypes

for _p in ("/opt/trn_rl_repo",):
    if _p not in sys.path:
        sys.path.append(_p)

NUM_MODELS = 4096
W = 32
N = 131072
NCORES = 8
NSTACK = 128          # stacks per core (windows of 32 experts globally)
BANK = 512
SGMAXC = 512          # columns per super-group (one PSUM bank)
SGMAXS = 16           # stacks per super-group (indicator tile partitions)
WB128F = 144          # per-stack cols in wb128: l1 64 | viewA 64 | sig 4 | rgb 12
WB16F = 256           # per-stack cols in wb16: l0aug 128 | viewBaug 128
PIPE = 2              # SG emission interleave (must divide bank parity)

BF16 = ml_dtypes.bfloat16


def _plan(counts):
    """Global (core-independent) packing plan. Returns expert order, caps,
    colstart, w_tot, super-groups."""
    order = np.argsort(-counts, kind="stable")
    win = order.reshape(NSTACK, 32)                      # window k -> 32 experts
    caps = np.maximum(4, -(-counts[win].max(axis=1) // 4) * 4)
    assert caps.max() <= BANK
    colstart = np.concatenate([[0], np.cumsum(caps)]).astype(np.int64)
    w_tot = int(colstart[-1])
    sgs = []
    s0 = 0
    while s0 < NSTACK:
        s1 = s0
        tot = 0
        while s1 < NSTACK and s1 - s0 < SGMAXS and tot + caps[s1] <= SGMAXC:
            tot += caps[s1]
            s1 += 1
        sgs.append((s0, s1))
        s0 = s1
    # stack pairs within each SG for K-stacked l0/viewB matmuls
    pairs = []
    pair_of = {}
    for (s0_, s1_) in sgs:
        k = s0_
        while k < s1_:
            k2 = k + 1 if k + 1 < s1_ else -1
            pair_of[k] = (len(pairs), 0)
            if k2 >= 0:
                pair_of[k2] = (len(pairs), 1)
            pairs.append((k, k2))
            k += 2
    return order, win, caps, colstart, w_tot, sgs, pairs, pair_of


def _prep(x, model_indices, pts_w0, pts_b0, pts_w1, pts_b1,
          feat_w, feat_b, sigma_w, sigma_b, view_w, view_b, rgb_w, rgb_b):
    x = np.asarray(x, np.float32)
    idx = np.asarray(model_indices).astype(np.int64)
    counts = np.bincount(idx, minlength=NUM_MODELS)
    order, win, caps, colstart, w_tot, sgs, pairs, pair_of = _plan(counts)
    nsg = len(sgs)

    order_pts = np.argsort(idx, kind="stable")
    starts = np.concatenate([[0], np.cumsum(counts)])

    pts_w0 = np.asarray(pts_w0, np.float32)
    pts_b0 = np.asarray(pts_b0, np.float32)
    pts_w1 = np.asarray(pts_w1, np.float32)
    pts_b1 = np.asarray(pts_b1, np.float32)
    feat_w = np.asarray(feat_w, np.float32)
    feat_b = np.asarray(feat_b, np.float32)
    sigma_w = np.asarray(sigma_w, np.float32)
    view_w = np.asarray(view_w, np.float32)
    rgb_w = np.asarray(rgb_w, np.float32)

    # fold feat layer into view layer (baseline trick):
    # view(h) = relu(Wv[:, :32] Wf h + Wv[:, 32:] views + bv + Wv[:, :32] bf)
    vwA = np.einsum("gox,gxh->goh", view_w[:, :, :W], feat_w)   # [E, o, h]
    vb_fold = view_b + np.einsum("goh,gh->go", view_w[:, :, :W], feat_b)

    w0aug = np.concatenate(
        [np.transpose(pts_w0, (0, 2, 1)), pts_b0[:, None, :]], axis=1
    )                                                   # [E, 4(kin), 32]
    vwBaug = np.concatenate(
        [np.transpose(view_w[:, :, W:], (0, 2, 1)), vb_fold[:, None, :]], axis=1
    )                                                   # [E, 4(kin), 32]
    w1T = np.transpose(pts_w1, (0, 2, 1))               # [E, a(in), h(out)]
    vwAT = np.transpose(vwA, (0, 2, 1))                 # [E, h(in), o(out)]

    # map stack index -> sg index and local row
    sg_of = np.empty(NSTACK, np.int64)
    loc_of = np.empty(NSTACK, np.int64)
    for g, (s0, s1) in enumerate(sgs):
        sg_of[s0:s1] = g
        loc_of[s0:s1] = np.arange(s1 - s0)

    per_core = []
    decode = []   # (core, stack, i, expert, cnt, col)
    for c in range(NCORES):
        gq = win[:, 4 * c:4 * c + 4]                    # [128, 4]

        xpts = np.zeros((32, w_tot), np.float32)
        views = np.zeros((32, w_tot), np.float32)
        ind = np.zeros((16, w_tot), np.float32)
        b1tab = np.zeros((16, 128 * nsg), np.float32)
        wb128 = np.zeros((128, WB128F * NSTACK), np.float32)
        wb16 = np.zeros((32, WB16F * len(pairs)), np.float32)

        for k in range(NSTACK):
            col = int(colstart[k])
            C = int(caps[k])
            g, loc = int(sg_of[k]), int(loc_of[k])
            ind[loc, col:col + C] = 1.0
            b1tab[loc, 128 * g:128 * (g + 1)] = pts_b1[gq[k]].reshape(128)
            w0 = WB128F * k
            pi, half = pair_of[k]
            s0 = WB16F * pi
            hb = 16 * half
            for i in range(4):
                e = int(gq[k, i])
                r = slice(32 * i, 32 * i + 32)
                # [64,64] half-blocks: expert i sits at rows 32i, block col
                # 32*(i%2) of half i//2; halves share the same 64 columns
                bc = 32 * (i % 2)
                wb128[r, w0 + bc:w0 + bc + 32] = w1T[e]
                wb128[r, w0 + 64 + bc:w0 + 96 + bc] = vwAT[e]
                wb128[r, w0 + 128 + i] = sigma_w[e, 0]
                wb128[r, w0 + 132 + 3 * i:w0 + 135 + 3 * i] = rgb_w[e].T
                wb16[hb + 4 * i:hb + 4 * i + 4,
                     s0 + 32 * i:s0 + 32 * i + 32] = w0aug[e]
                wb16[hb + 4 * i:hb + 4 * i + 4,
                     s0 + 128 + 32 * i:s0 + 160 + 32 * i] = vwBaug[e]
                cnt = int(counts[e])
                pts = order_pts[starts[e]:starts[e] + cnt]
                if cnt:
                    xv = x[pts]
                    xpts[hb + 4 * i:hb + 4 * i + 3, col:col + cnt] = xv[:, :3].T
                    views[hb + 4 * i:hb + 4 * i + 3, col:col + cnt] = \
                        xv[:, 3:6].T
                xpts[hb + 4 * i + 3, col:col + C] = 1.0
                views[hb + 4 * i + 3, col:col + C] = 1.0
                decode.append((c, k, i, e, cnt, col))

        per_core.append(dict(
            xpts=xpts.astype(BF16), views=views.astype(BF16),
            ind=ind.astype(BF16), b1tab=b1tab.astype(BF16),
            wb128=wb128.astype(BF16), wb16=wb16.astype(BF16)))

    return (per_core, decode, order_pts, starts, caps, colstart,
            w_tot, sgs, pairs, pair_of)


def _build_nc(caps, colstart, w_tot, sgs, pairs, pair_of):
    import concourse.mybir as mybir
    import concourse.tile as tile
    from concourse import bacc
    from contextlib import ExitStack

    f32 = mybir.dt.float32
    bf16 = mybir.dt.bfloat16
    RELU = mybir.ActivationFunctionType.Relu
    nsg = len(sgs)

    nc = bacc.Bacc("TRN2", target_bir_lowering=False)
    xpts_d = nc.declare_dram_parameter("xpts", [32, w_tot], bf16, isOutput=False)
    views_d = nc.declare_dram_parameter("views", [32, w_tot], bf16, isOutput=False)
    ind_d = nc.declare_dram_parameter("ind", [16, w_tot], bf16, isOutput=False)
    b1_d = nc.declare_dram_parameter("b1tab", [16, 128 * nsg], bf16, isOutput=False)
    wb128_d = nc.declare_dram_parameter("wb128", [128, WB128F * NSTACK], bf16,
                                        isOutput=False)
    wb16_d = nc.declare_dram_parameter("wb16", [32, WB16F * len(pairs)], bf16,
                                       isOutput=False)
    out_d = nc.declare_dram_parameter("out", [16, w_tot], f32, isOutput=True)

    with tile.TileContext(nc) as tc, ExitStack() as ctx:
        const = ctx.enter_context(tc.tile_pool(name="const", bufs=1))
        hpool = ctx.enter_context(tc.tile_pool(name="h", bufs=6))
        pspool = ctx.enter_context(tc.tile_pool(name="ps", bufs=1, space="PSUM"))

        xt = const.tile([32, w_tot], bf16)
        vt = const.tile([32, w_tot], bf16)
        it = const.tile([16, w_tot], bf16)
        bt = const.tile([16, 128 * nsg], bf16)
        wt128 = const.tile([128, WB128F * NSTACK], bf16)
        wt16 = const.tile([32, WB16F * len(pairs)], bf16)
        so = const.tile([44, w_tot], f32)
        ps = pspool.tile([128, 8 * BANK], f32, tag="ps")

        # Input DMAs: all issued up front (no deps), in consumption order,
        # alternating the SP and ACT HWDGE rings so transfers run on two
        # queues in parallel. Small first chunks let compute start early.
        # Both sequencers are otherwise idle at kernel start, so the issue
        # cost never delays compute ops.
        chunks = [(0, 1), (1, 2), (2, 3), (3, 4), (4, 6), (6, 8), (8, nsg)]
        chunks = [(a, min(b, nsg)) for a, b in chunks if a < min(b, nsg)]
        qs = [nc.sync, nc.scalar]
        qi = 0

        def pair_range(g0, g1):
            s0, s1 = sgs[g0][0], sgs[g1 - 1][1]
            return pair_of[s0][0], pair_of[s1 - 1][0] + 1

        # the very first l0 matmul needs only SG0's xt + wt16 slices; put
        # those tiny DMAs first so the pipeline starts right after the
        # framework preamble instead of behind the first weight chunk
        fs0, fs1 = sgs[0]
        flo, fhi = int(colstart[fs0]), int(colstart[fs1])
        fp0, fp1 = pair_range(0, 1)
        nc.sync.dma_start(out=xt[:, flo:fhi], in_=xpts_d[:, flo:fhi])
        nc.scalar.dma_start(out=wt16[:, WB16F * fp0:WB16F * fp1],
                            in_=wb16_d[:, WB16F * fp0:WB16F * fp1])
        nc.gpsimd.dma_start(out=bt, in_=b1_d[:, :])
        for g0, g1 in chunks:
            s0, s1 = sgs[g0][0], sgs[g1 - 1][1]
            lo, hi = int(colstart[s0]), int(colstart[s1])
            p0, p1 = pair_range(g0, g1)
            first = g0 == 0
            # wb128 (9.4MB) split: ~45% sync, ~45% scalar, ~10% gpsimd so
            # all three rings (~112/112/85 GB/s) finish together with the
            # small tensors also on the gpsimd SWDGE ring.
            if g0 < 3:
                m1 = s0 + max(1, (s1 - s0 + 1) // 2)
                m2 = s1
            else:
                m1 = s0 + max(1, round((s1 - s0) * 0.45))
                m2 = min(s1, m1 + max(1, round((s1 - s0) * 0.45)))
            qa, qb = qs[qi % 2], qs[(qi + 1) % 2]
            qi += 1
            qa.dma_start(out=wt128[:, WB128F * s0:WB128F * m1],
                         in_=wb128_d[:, WB128F * s0:WB128F * m1])
            qb.dma_start(out=wt128[:, WB128F * m1:WB128F * m2],
                         in_=wb128_d[:, WB128F * m1:WB128F * m2])
            if m2 < s1:
                nc.gpsimd.dma_start(out=wt128[:, WB128F * m2:WB128F * s1],
                                    in_=wb128_d[:, WB128F * m2:WB128F * s1])
            if not first:
                nc.gpsimd.dma_start(out=wt16[:, WB16F * p0:WB16F * p1],
                                    in_=wb16_d[:, WB16F * p0:WB16F * p1])
                nc.gpsimd.dma_start(out=xt[:, lo:hi], in_=xpts_d[:, lo:hi])
            nc.gpsimd.dma_start(out=it[:, lo:hi], in_=ind_d[:, lo:hi])
            nc.gpsimd.dma_start(out=vt[:, lo:hi], in_=views_d[:, lo:hi])

        def sg_steps(g):
            s0, s1 = sgs[g]
            Ssg = s1 - s0
            lo = int(colstart[s0])
            sgw = int(colstart[s1]) - lo
            # Role-shared banks: l0/view share a bank (disjoint lifetimes:
            # l0->relu1 then view->relu3), as do l1 and sigma/rgb. 2 banks
            # per SG x 4-deep parity = 8 banks, a 4-SG pipeline.
            p = g % 4
            psA = ps[:, p * BANK:p * BANK + sgw]
            psB = ps[:, (4 + p) * BANK:(4 + p) * BANK + sgw]
            psC = psA
            psD = psB
            state = {}

            def cw(k):
                a = int(colstart[k]) - lo
                return a, a + int(caps[k])

            sg_pairs = [pairs[pi] for pi in
                        range(pair_of[s0][0], pair_of[s1 - 1][0] + 1)]

            def pw(pr):
                k, k2 = pr
                a = int(colstart[k]) - lo
                b = (int(colstart[k2]) + int(caps[k2]) - lo
                     if k2 >= 0 else a + int(caps[k]))
                return a, b

            def s_l0():
                for n, pr in enumerate(sg_pairs):
                    pi = pair_of[pr[0]][0]
                    a, b = pw(pr)
                    nc.tensor.matmul(
                        out=psA[:, a:b],
                        lhsT=wt16[:, WB16F * pi:WB16F * pi + 128],
                        rhs=xt[:, lo + a:lo + b],
                        start=(n == 0), stop=(n == len(sg_pairs) - 1),
                        skip_group_check=True)

            def s_relu1():
                h1 = hpool.tile([128, SGMAXC], bf16, tag="h1")
                nc.scalar.activation(h1[:, 0:sgw], psA, RELU)
                state["h1"] = h1

            def s_l1():
                h1 = state.pop("h1")
                # l1 matmuls arm the bank (zero-fill-on-write covers every
                # byte); the bias matmul accumulates LAST so its it/b1tab
                # DMA dependencies never block the l1 weights at the PE
                # queue head.
                for k in range(s0, s1):
                    a, b = cw(k)
                    for hf in range(2):
                        nc.tensor.matmul(
                            out=psB[64 * hf:64 * hf + 64, a:b],
                            lhsT=wt128[64 * hf:64 * hf + 64,
                                       WB128F * k:WB128F * k + 64],
                            rhs=h1[64 * hf:64 * hf + 64, a:b],
                            start=(k == s0), stop=False,
                            skip_group_check=True,
                            tile_position=(64 * hf, 64 * hf))
                nc.tensor.matmul(
                    out=psB,
                    lhsT=bt[0:Ssg, 128 * g:128 * (g + 1)],
                    rhs=it[0:Ssg, lo:lo + sgw],
                    start=False, stop=True, skip_group_check=True)

            def s_relu2():
                h2 = hpool.tile([128, SGMAXC], bf16, tag="h2")
                nc.vector.tensor_scalar_max(h2[:, 0:sgw], psB, 0.0)
                state["h2"] = h2

            def s_sigview():
                h2 = state.pop("h2")
                for k in range(s0, s1):
                    a, b = cw(k)
                    nc.tensor.matmul(
                        out=psD[0:4, a:b],
                        lhsT=wt128[:, WB128F * k + 128:WB128F * k + 132],
                        rhs=h2[:, a:b],
                        start=(k == s0), stop=(k == s1 - 1),
                        skip_group_check=True)
                for k in range(s0, s1):
                    a, b = cw(k)
                    for hf in range(2):
                        nc.tensor.matmul(
                            out=psC[64 * hf:64 * hf + 64, a:b],
                            lhsT=wt128[64 * hf:64 * hf + 64,
                                       WB128F * k + 64:WB128F * k + 128],
                            rhs=h2[64 * hf:64 * hf + 64, a:b],
                            start=(k == s0), stop=False,
                            skip_group_check=True,
                            tile_position=(64 * hf, 64 * hf))
                for n, pr in enumerate(sg_pairs):
                    pi = pair_of[pr[0]][0]
                    a, b = pw(pr)
                    nc.tensor.matmul(
                        out=psC[:, a:b],
                        lhsT=wt16[:, WB16F * pi + 128:WB16F * pi + 256],
                        rhs=vt[:, lo + a:lo + b],
                        start=False, stop=(n == len(sg_pairs) - 1),
                        skip_group_check=True)

            def s_relu3():
                hv = hpool.tile([128, SGMAXC], bf16, tag="hv")
                nc.scalar.activation(hv[:, 0:sgw], psC, RELU)
                state["hv"] = hv

            def s_rgb():
                hv = state.pop("hv")
                for k in range(s0, s1):
                    a, b = cw(k)
                    nc.tensor.matmul(
                        out=psD[32:44, a:b],
                        lhsT=wt128[:, WB128F * k + 132:WB128F * k + 144],
                        rhs=hv[:, a:b],
                        start=(k == s0), stop=(k == s1 - 1),
                        skip_group_check=True)

            def s_out():
                nc.vector.tensor_copy(so[32:44, lo:lo + sgw], psD[32:44, :])
                nc.vector.tensor_copy(so[0:4, lo:lo + sgw], psD[0:4, :])
                nc.sync.dma_start(out=out_d[0:12, lo:lo + sgw],
                                  in_=so[32:44, lo:lo + sgw])
                nc.sync.dma_start(out=out_d[12:16, lo:lo + sgw],
                                  in_=so[0:4, lo:lo + sgw])

            return [s_l0, s_relu1, s_l1, s_relu2, s_sigview, s_relu3,
                    s_rgb, s_out]

        # Skewed modulo schedule: at tick t the PE runs l0(t), l1(t-1),
        # sigview(t-2), rgb(t-3), so each step's relu finished a full tick
        # (~3us of PE work) before its consumer issues. PSUM parity-2
        # lifetimes are exactly 2 ticks, matching the bank assignment.
        steps = [sg_steps(g) for g in range(nsg)]
        skew = [0, 0, 1, 1, 2, 2, 3, 3]
        for t in range(nsg + 3):
            for j in range(8):
                g = t - skew[j]
                if 0 <= g < nsg:
                    steps[g][j]()

    nc.compile()
    return nc


def _finish(results, decode, order_pts, starts, sigma_b, rgb_b):
    y = np.empty((N, 4), np.float32)
    outs = [np.asarray(r["out"]) for r in results]
    for (c, k, i, e, cnt, col) in decode:
        if cnt == 0:
            continue
        o = outs[c]
        pts = order_pts[starts[e]:starts[e] + cnt]
        y[pts, 0:3] = o[3 * i:3 * i + 3, col:col + cnt].T + rgb_b[e]
        y[pts, 3] = o[12 + i, col:col + cnt] + sigma_b[e, 0]
    return y


def kernel(**inputs):
    from concourse.bass_utils import run_bass_kernel_spmd

    (per_core, decode, order_pts, starts, caps, colstart, w_tot, sgs,
     pairs, pair_of) = _prep(**inputs)
    nc = _build_nc(caps, colstart, w_tot, sgs, pairs, pair_of)
    res = run_bass_kernel_spmd(nc, per_core, list(range(NCORES)))
    return _finish(res.results, decode, order_pts, starts,
                   np.asarray(inputs["sigma_b"], np.float32),
                   np.asarray(inputs["rgb_b"], np.float32))


# ---------------------------------------------------------------------------
# numpy emulation of the device program (layout validation)
def _emulate_core(arrs, caps, colstart, w_tot, sgs, pairs, pair_of):
    f = np.float32
    xt = arrs["xpts"].astype(f)
    vt = arrs["views"].astype(f)
    it = arrs["ind"].astype(f)
    bt = arrs["b1tab"].astype(f)
    wt128 = arrs["wb128"].astype(f)
    wt16 = arrs["wb16"].astype(f)
    out = np.zeros((16, w_tot), f)
    for g, (s0, s1) in enumerate(sgs):
        Ssg = s1 - s0
        lo = int(colstart[s0])
        sgw = int(colstart[s1]) - lo
        psA = np.zeros((128, sgw), f)
        psB = np.zeros((128, sgw), f)
        psC = np.zeros((128, sgw), f)
        psD = np.zeros((16, sgw), f)
        k = s0
        while k < s1:
            pi, _ = pair_of[k]
            kk, k2 = pairs[pi]
            a = int(colstart[kk]) - lo
            b = (int(colstart[k2]) + int(caps[k2]) - lo
                 if k2 >= 0 else a + int(caps[kk]))
            psA[:, a:b] = wt16[:, WB16F * pi:WB16F * pi + 128].T @ \
                xt[:, lo + a:lo + b]
            k = k2 + 1 if k2 >= 0 else k + 1
        h1 = np.maximum(psA, 0).astype(BF16).astype(f)
        psB[:] = bt[0:Ssg, 128 * g:128 * (g + 1)].T @ it[0:Ssg, lo:lo + sgw]
        for k in range(s0, s1):
            a = int(colstart[k]) - lo
            b = a + int(caps[k])
            for hf in range(2):
                r = slice(64 * hf, 64 * hf + 64)
                psB[r, a:b] += \
                    wt128[r, WB128F * k:WB128F * k + 64].T @ h1[r, a:b]
        h2 = np.maximum(psB, 0).astype(BF16).astype(f)
        for k in range(s0, s1):
            a = int(colstart[k]) - lo
            b = a + int(caps[k])
            psD[0:4, a:b] = \
                wt128[:, WB128F * k + 128:WB128F * k + 132].T @ h2[:, a:b]
            pi, _ = pair_of[k]
            psC[:, a:b] = wt16[:, WB16F * pi + 128:WB16F * pi + 256].T @ \
                vt[:, lo + a:lo + b]
            for hf in range(2):
                r = slice(64 * hf, 64 * hf + 64)
                psC[r, a:b] += \
                    wt128[r, WB128F * k + 64:WB128F * k + 128].T @ h2[r, a:b]
        hv = np.maximum(psC, 0).astype(BF16).astype(f)
        for k in range(s0, s1):
            a = int(colstart[k]) - lo
            b = a + int(caps[k])
            psD[4:16, a:b] = \
                wt128[:, WB128F * k + 132:WB128F * k + 144].T @ hv[:, a:b]
        out[0:12, lo:lo + sgw] = psD[4:16]
        out[12:16, lo:lo + sgw] = psD[0:4]
    return out


def kernel_emulated(**inputs):
    (per_core, decode, order_pts, starts, caps, colstart, w_tot, sgs,
     pairs, pair_of) = _prep(**inputs)
    results = [{"out": _emulate_core(per_core[c], caps, colstart, w_tot, sgs,
                                     pairs, pair_of)}
               for c in range(NCORES)]
    return _finish(results, decode, order_pts, starts,
                   np.asarray(inputs["sigma_b"], np.float32),
                   np.asarray(inputs["rgb_b"], np.float32))
